# revision 1
# baseline (speedup 1.0000x reference)
"""Trainium2 Bass kernel for AttnLRP multi-head attention forward.

Reference computation (forward only; divide_grad is identity in fwd):
    qkv = x @ w_qkv.T + b_qkv            # [B,N,3C]
    q,k,v = split/reshape -> [B,H,N,D]
    attn = softmax(q*D^-0.5 @ k^T)       # [B,H,N,N]
    out  = (attn @ v) reshaped -> [B,N,C]
    out  = out @ w_proj.T + b_proj

Shapes: B=4, N=2048, C=1024, H=16, D=64.

Sharding over 8 NeuronCores: tensor-parallel over heads. Core c owns heads
{2c, 2c+1} for all batches (column-parallel qkv, row-parallel proj). Each core
emits a partial projection output [B*N, C]; the host sums the 8 partials and
adds b_proj.

Per-core kernel works in a fully transposed layout:
  qT/kT [128=2*64 d-channels, 8192 tokens], v in natural token-major layout
  (via PE transposes) augmented with a ones column so the attention-value
  matmul also produces the softmax denominators (row 64 of the PSUM tile).
Scores are computed per 128-key chunk as S^T [keys, queries] with the two
heads packed into the two K=64 row-groups of the PE array; exp runs as a
single wide ScalarE activation over both heads' PSUM banks; AV accumulates
over key chunks in PSUM. All matmuls run in float32r (full fp32 storage,
fast PE mode).
"""

import os
import sys

sys.path.insert(0, "/opt/trn_rl_repo")

import numpy as np

import concourse.bass as bass
import concourse.tile as tile
from concourse import bacc, mybir
from concourse.bass_utils import run_bass_kernel_spmd
from concourse.masks import make_identity

B, N, C = 4, 2048, 1024
H, D = 16, 64
NCORES = 8
BN = B * N  # 8192 tokens total
HPC = H // NCORES  # 2 heads per core
CHPC = HPC * D  # 128 channels per core
SCALE = D ** -0.5
F32 = mybir.dt.float32
F32R = mybir.dt.float32r

TOK_TILE = 512  # token tile for qkv projection / query tile for attention
N_TOK_TILES = BN // TOK_TILE  # 16
KO = C // 128  # 8 contraction chunks for qkv projection
MC = N // 128  # 16 key chunks per batch
NT = N // TOK_TILE  # 4 query tiles per batch


def round_fp32r(a):
    """Round fp32 array to the fp32r domain: E8M11, round-to-nearest-even,
    low 12 mantissa bits zero (matches walrus fp32_to_fp32r)."""
    b = np.ascontiguousarray(a, dtype=np.float32).view(np.uint32).copy()
    low = b & np.uint32(0xFFF)
    lsb = (b >> np.uint32(12)) & np.uint32(1)
    round_up = (low > 0x800) | ((low == 0x800) & (lsb == 1))
    b = (b & ~np.uint32(0xFFF)) + (round_up.astype(np.uint32) << np.uint32(12))
    return b.view(np.float32)


def build_program():
    nc = bacc.Bacc("TRN2", debug=False, num_devices=NCORES)

    xT = nc.dram_tensor("xT", [C, BN], F32R, kind="ExternalInput").ap()
    wT = nc.dram_tensor("wT", [C, 3 * CHPC], F32R, kind="ExternalInput").ap()
    bqkv = nc.dram_tensor("bqkv", [3 * CHPC], F32, kind="ExternalInput").ap()
    wpT = nc.dram_tensor("wpT", [CHPC, C], F32R, kind="ExternalInput").ap()
    out = nc.dram_tensor("out", [BN, C], F32, kind="ExternalOutput").ap()

    xT3 = xT.rearrange("(ko p) n -> p ko n", p=128)  # [128, 8, 8192]
    wT3 = wT.rearrange("(ko p) m -> p ko m", p=128)  # [128, 8, 384]
    b2 = bqkv.rearrange("(blk p) -> p blk", p=128)  # [128, 3]

    with tile.TileContext(nc) as tc:
        with (
            tc.tile_pool(name="singles", bufs=1) as singles,
            tc.tile_pool(name="xin", bufs=3) as xin,
            tc.tile_pool(name="vstage", bufs=2) as vstage_pool,
            tc.tile_pool(name="pt", bufs=3) as ptpool,
            tc.tile_pool(name="attnw", bufs=2) as attnpool,
            tc.tile_pool(name="outsb", bufs=2) as outsb_pool,
            tc.tile_pool(name="small", bufs=2) as small,
            tc.tile_pool(name="sps", bufs=2, space="PSUM") as sps,
            tc.tile_pool(name="avps", bufs=2, space="PSUM") as avps,
        ):
            # --- resident tensors ---
            wT_sb = singles.tile([128, KO, 3 * CHPC], F32R)
            nc.sync.dma_start(wT_sb[:], wT3[:])
            bias_sb = singles.tile([128, 3], F32)
            nc.sync.dma_start(bias_sb[:], b2[:])
            wpT_sb = singles.tile([128, C], F32R)
            nc.sync.dma_start(wpT_sb[:], wpT[:])
            ident = singles.tile([128, 128], F32)
            make_identity(nc, ident)

            qT = singles.tile([128, BN], F32R)
            kT = singles.tile([128, BN], F32R)
            # v in token-major layout per batch: [tok_part, batch, key_chunk,
            # 130] where cols 0:64 = head A, 64 = ones, 65:129 = head B,
            # 129 = ones.
            # v in token-major layout: cols [0:64]=head A, [64]=ones,
            # [66:130]=head B, [130]=ones (65/131 pad). Each head's AV lhsT is
            # [ch(64), ones] so channels land at PSUM rows 0:64 and the
            # softmax denominator at row 64 (32-aligned slices only).
            v_aug = singles.tile([128, B, MC, 132], F32R)
            ones_sb = singles.tile([128, 1], F32)
            nc.vector.memset(ones_sb[:], 1.0)
            ones_bc = ones_sb[:, None, None, :].to_broadcast((128, B, MC, 1))
            nc.vector.tensor_copy(out=v_aug[:, :, :, 64:65], in_=ones_bc)
            nc.vector.tensor_copy(out=v_aug[:, :, :, 130:131], in_=ones_bc)

            pending_proj = []

            def emit_proj(keep_last):
                # projection for 512 tokens (row-parallel partial); PSUM via
                # the av tags (transient allocs between AV accumulations)
                while len(pending_proj) > (1 if keep_last else 0):
                    attn_w, q0 = pending_proj.pop(0)
                    for st in range(TOK_TILE // 128):
                        osb = outsb_pool.tile([128, C], F32, tag="osb")
                        for half in range(2):
                            pp = avps.tile(
                                [128, 512], F32, tag="avA" if half == 0 else "avB"
                            )
                            nc.tensor.matmul(
                                pp[:],
                                lhsT=attn_w[:, st * 128 : st * 128 + 128],
                                rhs=wpT_sb[:, half * 512 : half * 512 + 512],
                                start=True,
                                stop=True,
                            )
                            nc.vector.tensor_copy(
                                out=osb[:, half * 512 : half * 512 + 512], in_=pp[:]
                            )
                        tok0 = q0 + st * 128
                        # output writes go out the GpSimd (SWDGE) queue so they
                        # don't contend with x-tile prefetches on the sync ring
                        nc.gpsimd.dma_start(out[tok0 : tok0 + 128, :], osb[:])

            def load_x_tile(b, tt4):
                """Issue the two 1MB x-half DMAs for a 512-token tile."""
                t0 = b * N + tt4 * TOK_TILE
                xts = []
                for half in range(2):
                    xt = xin.tile([128, KO // 2, TOK_TILE], F32R, tag="xt")
                    nc.sync.dma_start(
                        xt[:], xT3[:, half * 4 : half * 4 + 4, t0 : t0 + TOK_TILE]
                    )
                    xts.append(xt)
                return xts

            def emit_qkv_tile(b, tt4, use_av_tags, xts=None):
                """qkv projection + v transposition for one 512-token tile.

                use_av_tags=False: standalone (prologue) — accumulate in "sp"
                slots. use_av_tags=True: interleaved into an attention block —
                accumulate q/k then v in the free "av" bank set (two passes)
                so the score pipeline keeps its "sp" slots.
                """
                t0 = b * N + tt4 * TOK_TILE
                if xts is None:
                    xts = load_x_tile(b, tt4)

                def xchunk(ko):
                    return xts[ko // 4][:, ko % 4, :]

                if use_av_tags:
                    psq = avps.tile([128, TOK_TILE], F32, tag="avA")
                    psk = avps.tile([128, TOK_TILE], F32, tag="avB")
                else:
                    ps_qk = sps.tile([128, 2 * TOK_TILE], F32, tag="sp")
                    psq = ps_qk[:, 0:TOK_TILE]
                    psk = ps_qk[:, TOK_TILE : 2 * TOK_TILE]
                for ko in range(KO):
                    for blk, ps in ((0, psq), (1, psk)):
                        nc.tensor.matmul(
                            ps,
                            lhsT=wT_sb[:, ko, blk * 128 : blk * 128 + 128],
                            rhs=xchunk(ko),
                            start=(ko == 0),
                            stop=(ko == KO - 1),
                        )
                nc.vector.tensor_scalar_add(
                    qT[:, t0 : t0 + TOK_TILE], psq, bias_sb[:, 0:1]
                )
                nc.vector.tensor_scalar_add(
                    kT[:, t0 : t0 + TOK_TILE], psk, bias_sb[:, 1:2]
                )
                if use_av_tags:
                    psv = avps.tile([128, TOK_TILE], F32, tag="avA")
                else:
                    ps_v = sps.tile([128, 2 * TOK_TILE], F32, tag="sp")
                    psv = ps_v[:, 0:TOK_TILE]
                for ko in range(KO):
                    nc.tensor.matmul(
                        psv,
                        lhsT=wT_sb[:, ko, 256 : 256 + 128],
                        rhs=xchunk(ko),
                        start=(ko == 0),
                        stop=(ko == KO - 1),
                    )
                vst = vstage_pool.tile([128, TOK_TILE], F32, tag="vst")
                nc.vector.tensor_scalar_add(vst[:], psv, bias_sb[:, 2:3])
                for st in range(TOK_TILE // 128):
                    mc_idx = tt4 * (TOK_TILE // 128) + st
                    pst = avps.tile(
                        [128, 512], F32, tag="avB" if st % 2 == 0 else "avA"
                    )
                    nc.tensor.transpose(
                        pst[:, 0:128], vst[:, st * 128 : st * 128 + 128], ident[:]
                    )
                    nc.scalar.copy(out=v_aug[:, b, mc_idx, 0:64], in_=pst[:, 0:64])
                    nc.scalar.copy(
                        out=v_aug[:, b, mc_idx, 66:130], in_=pst[:, 64:128]
                    )

            # --- schedule: batch-0 qkv prologue, then attention blocks with
            # the next batch's qkv tiles interleaved as PE filler ---
            for tt4 in range(N // TOK_TILE):
                emit_qkv_tile(0, tt4, use_av_tags=False)

            for b in range(B):
                for nt in range(NT):
                    # issue the interleaved qkv tile's x loads up front so the
                    # DMA completes during this block's m-loop
                    xts = load_x_tile(b + 1, nt) if b + 1 < B else None
                    q0 = b * N + nt * TOK_TILE  # global query offset
                    avA = avps.tile([65, TOK_TILE], F32, tag="avA")
                    avB = avps.tile([65, TOK_TILE], F32, tag="avB")
                    for mc in range(MC):
                        m0 = b * N + mc * 128  # global key offset
                        sp = sps.tile([128, 2 * TOK_TILE], F32, tag="sp")
                        nc.tensor.matmul(
                            sp[:, 0:TOK_TILE],
                            lhsT=kT[0:64, m0 : m0 + 128],
                            rhs=qT[0:64, q0 : q0 + TOK_TILE],
                            start=True,
                            stop=True,
                        )
                        nc.tensor.matmul(
                            sp[:, TOK_TILE : 2 * TOK_TILE],
                            lhsT=kT[64:128, m0 : m0 + 128],
                            rhs=qT[64:128, q0 : q0 + TOK_TILE],
                            start=True,
                            stop=True,
                        )
                        pt = ptpool.tile([128, 2 * TOK_TILE], F32R, tag="pt")
                        nc.scalar.activation(
                            pt[:], sp[:], mybir.ActivationFunctionType.Exp
                        )
                        nc.tensor.matmul(
                            avA[:],
                            lhsT=v_aug[:, b, mc, 0:65],
                            rhs=pt[:, 0:TOK_TILE],
                            start=(mc == 0),
                            stop=(mc == MC - 1),
                        )
                        nc.tensor.matmul(
                            avB[:],
                            lhsT=v_aug[:, b, mc, 66:131],
                            rhs=pt[:, TOK_TILE : 2 * TOK_TILE],
                            start=(mc == 0),
                            stop=(mc == MC - 1),
                        )

                    # stage AV out of PSUM immediately (frees the banks for
                    # the next query tile); both heads: channels rows 0:64,
                    # denominator row 64. Both denominators go into one
                    # [33, 512] tile (rows 0 and 32) for a single reciprocal.
                    avstA = small.tile([65, TOK_TILE], F32, tag="avstA")
                    avstB = small.tile([65, TOK_TILE], F32, tag="avstB")
                    nc.vector.tensor_copy(out=avstA[:], in_=avA[:])
                    nc.scalar.copy(out=avstB[:], in_=avB[:])
                    s33 = small.tile([33, TOK_TILE], F32, tag="s33")
                    nc.vector.memset(s33[:], 1.0)
                    nc.vector.tensor_copy(out=s33[0:1, :], in_=avstA[64:65, :])
                    nc.vector.tensor_copy(out=s33[32:33, :], in_=avstB[64:65, :])
                    r33 = small.tile([33, TOK_TILE], F32, tag="r33")
                    nc.vector.reciprocal(r33[:], s33[:])
                    # partition_broadcast requires a partition-0 source on HW
                    # (a base-32 AP silently broadcasts garbage), so bounce
                    # head B's reciprocal row through a base-0 tile.
                    rB0 = small.tile([1, TOK_TILE], F32, tag="rB0")
                    nc.vector.tensor_copy(out=rB0[:], in_=r33[32:33, :])
                    rbA = small.tile([64, TOK_TILE], F32, tag="rbA")
                    rbB = small.tile([64, TOK_TILE], F32, tag="rbB")
                    nc.gpsimd.partition_broadcast(rbA[:], r33[0:1, :])
                    nc.gpsimd.partition_broadcast(rbB[:], rB0[:])
                    attn_w = attnpool.tile([128, TOK_TILE], F32R, tag="attnw")
                    nc.vector.tensor_tensor(
                        attn_w[0:64, :], avstA[0:64, :], rbA[:], mybir.AluOpType.mult
                    )
                    nc.vector.tensor_tensor(
                        attn_w[64:128, :], avstB[0:64, :], rbB[:], mybir.AluOpType.mult
                    )
                    pending_proj.append((attn_w, q0))

                    emit_proj(keep_last=True)
                    if b + 1 < B:
                        emit_qkv_tile(b + 1, nt, use_av_tags=True, xts=xts)

            emit_proj(keep_last=False)

    nc.compile()
    return nc


_NC = None


def _get_nc():
    global _NC
    if _NC is None:
        _NC = build_program()
    return _NC


def make_in_maps(x, w_qkv, b_qkv, w_proj):
    x = np.asarray(x, dtype=np.float32)
    w_qkv = np.asarray(w_qkv, dtype=np.float32)
    b_qkv = np.asarray(b_qkv, dtype=np.float32)
    w_proj = np.asarray(w_proj, dtype=np.float32)

    xT = round_fp32r(x.reshape(BN, C).T)  # [C, BN], fp32r domain
    in_maps = []
    for c in range(NCORES):
        r0 = c * CHPC
        wq = w_qkv[r0 : r0 + CHPC] * SCALE
        wk = w_qkv[C + r0 : C + r0 + CHPC]
        wv = w_qkv[2 * C + r0 : 2 * C + r0 + CHPC]
        wT_c = round_fp32r(np.concatenate([wq, wk, wv], axis=0).T)
        b_c = np.concatenate(
            [
                b_qkv[r0 : r0 + CHPC] * SCALE,
                b_qkv[C + r0 : C + r0 + CHPC],
                b_qkv[2 * C + r0 : 2 * C + r0 + CHPC],
            ]
        ).astype(np.float32)
        wpT_c = round_fp32r(w_proj[:, r0 : r0 + CHPC].T)  # [CHPC, C]
        in_maps.append({"xT": xT, "wT": wT_c, "bqkv": b_c, "wpT": wpT_c})
    return in_maps


def kernel(x, w_qkv, b_qkv, w_proj, b_proj, _trace=False, _trace_kwargs=None):
    nc = _get_nc()
    in_maps = make_in_maps(x, w_qkv, b_qkv, w_proj)
    kwargs = {}
    if _trace:
        kwargs.update(trace=True, **(_trace_kwargs or {}))
    res = run_bass_kernel_spmd(nc, in_maps, core_ids=list(range(NCORES)), **kwargs)
    acc = res.results[0]["out"].astype(np.float32)
    for c in range(1, NCORES):
        acc = acc + res.results[c]["out"]
    acc = acc + np.asarray(b_proj, dtype=np.float32)[None, :]
    out = acc.reshape(B, N, C)
    kernel.last_results = res
    return out



# revision 14
# speedup vs baseline: 1.2365x; 1.2365x over previous
"""Trainium2 Bass kernel for AttnLRP multi-head attention forward.

Reference computation (forward only; divide_grad is identity in fwd):
    qkv = x @ w_qkv.T + b_qkv            # [B,N,3C]
    q,k,v = split/reshape -> [B,H,N,D]
    attn = softmax(q*D^-0.5 @ k^T)       # [B,H,N,N]
    out  = (attn @ v) reshaped -> [B,N,C]
    out  = out @ w_proj.T + b_proj

Shapes: B=4, N=2048, C=1024, H=16, D=64.

Sharding over 8 NeuronCores: tensor-parallel over heads. Core c owns heads
{2c, 2c+1} for all batches (column-parallel qkv, row-parallel proj). Each core
emits a partial projection output [B*N, C]; the host sums the 8 partials and
adds b_proj.

Per-core kernel works in a fully transposed layout:
  qT/kT [128=2*64 d-channels, 8192 tokens] fp16, v in natural token-major
  layout (via PE transposes) augmented with a ones column so the
  attention-value matmul also produces the softmax denominators (row 64 of the
  PSUM tile). Scores are computed per 128-key chunk as S^T [keys, queries]
  with the two heads packed into the two K=64 row-groups of the PE array
  (concurrent); exp runs as a single wide ScalarE activation over both heads'
  PSUM banks; AV accumulates over key chunks in PSUM. All matmul operands are
  fp16 (1 cycle/column on the PE), accumulation stays fp32 in PSUM.

The steady-state mc loop is ScalarE-bound (exp of [128,1024] per key chunk =
~1.1us vs ~0.65us of PE work), so the projection of the previous query tile
and the qkv projection of the next batch's tile are interleaved INTO the mc
loop as PE filler units (2 per key chunk) instead of running as serial bursts
after it.
"""

import os
import sys

sys.path.insert(0, "/opt/trn_rl_repo")

import numpy as np

import concourse.bass as bass
import concourse.tile as tile
from concourse import bacc, mybir
from concourse.bass_utils import run_bass_kernel_spmd
from concourse.masks import make_identity

B, N, C = 4, 2048, 1024
H, D = 16, 64
NCORES = 8
BN = B * N  # 8192 tokens total
HPC = H // NCORES  # 2 heads per core
CHPC = HPC * D  # 128 channels per core
SCALE = D ** -0.5
F32 = mybir.dt.float32
F16 = mybir.dt.float16

TOK_TILE = 512  # token tile for qkv projection / query tile for attention
KO = C // 128  # 8 contraction chunks for qkv projection
MC = N // 128  # 16 key chunks per batch
NT = N // TOK_TILE  # 4 query tiles per batch


def build_program():
    nc = bacc.Bacc("TRN2", debug=False, num_devices=NCORES)

    xT = nc.dram_tensor("xT", [C, BN], F16, kind="ExternalInput").ap()
    wT = nc.dram_tensor("wT", [C, 3 * CHPC], F16, kind="ExternalInput").ap()
    bqkv = nc.dram_tensor("bqkv", [3 * CHPC], F32, kind="ExternalInput").ap()
    wpT = nc.dram_tensor("wpT", [CHPC, C], F16, kind="ExternalInput").ap()
    out = nc.dram_tensor("out", [BN, C], F32, kind="ExternalOutput").ap()

    xT3 = xT.rearrange("(ko p) n -> p ko n", p=128)  # [128, 8, 8192]
    wT3 = wT.rearrange("(ko p) m -> p ko m", p=128)  # [128, 8, 384]
    b2 = bqkv.rearrange("(blk p) -> p blk", p=128)  # [128, 3]

    with tile.TileContext(nc) as tc:
        with (
            tc.tile_pool(name="singles", bufs=1) as singles,
            tc.tile_pool(name="xin", bufs=3) as xin,
            tc.tile_pool(name="vstage", bufs=2) as vstage_pool,
            tc.tile_pool(name="pt", bufs=3) as ptpool,
            tc.tile_pool(name="attnw", bufs=2) as attnpool,
            tc.tile_pool(name="outsb", bufs=2) as outsb_pool,
            tc.tile_pool(name="small", bufs=2) as small,
            tc.tile_pool(name="sps", bufs=2, space="PSUM") as sps,
            tc.tile_pool(name="avps", bufs=2, space="PSUM") as avps,
        ):
            # --- resident tensors ---
            wT_sb = singles.tile([128, KO, 3 * CHPC], F16)
            nc.sync.dma_start(wT_sb[:], wT3[:])
            bias_sb = singles.tile([128, 3], F32)
            nc.sync.dma_start(bias_sb[:], b2[:])
            wpT_sb = singles.tile([128, C], F16)
            nc.sync.dma_start(wpT_sb[:], wpT[:])
            ident = singles.tile([128, 128], F32)
            make_identity(nc, ident)

            qT = singles.tile([128, BN], F16)
            kT = singles.tile([128, BN], F16)
            # v in token-major layout: cols [0:64]=head A, [64]=ones,
            # [66:130]=head B, [130]=ones (65/131 pad). Each head's AV lhsT is
            # [ch(64), ones] so channels land at PSUM rows 0:64 and the
            # softmax denominator at row 64 (32-aligned slices only).
            v_aug = singles.tile([128, B, MC, 132], F16)
            ones_sb = singles.tile([128, 1], F16)
            nc.vector.memset(ones_sb[:], 1.0)
            ones_bc = ones_sb[:, None, None, :].to_broadcast((128, B, MC, 1))
            nc.vector.tensor_copy(out=v_aug[:, :, :, 64:65], in_=ones_bc)
            nc.vector.tensor_copy(out=v_aug[:, :, :, 130:131], in_=ones_bc)

            # --- filler-unit machinery: small closures, each ~1-2 PE matmuls,
            # popped two per mc iteration inside the attention loop ---
            filler = []

            def drain(k):
                for _ in range(k):
                    if filler:
                        filler.pop(0)()

            def drain_all():
                while filler:
                    filler.pop(0)()

            def push_proj(attn_w, q0):
                # projection of 512 tokens (row-parallel partial), split into
                # 4 token-subtile units of 2 matmuls each
                def unit(st):
                    def run():
                        osb = outsb_pool.tile([128, C], F32, tag="osb")
                        for half in range(2):
                            pp = avps.tile(
                                [128, 512], F32, tag="avA" if half == 0 else "avB"
                            )
                            nc.tensor.matmul(
                                pp[:],
                                lhsT=attn_w[:, st * 128 : st * 128 + 128],
                                rhs=wpT_sb[:, half * 512 : half * 512 + 512],
                                start=True,
                                stop=True,
                            )
                            nc.vector.tensor_copy(
                                out=osb[:, half * 512 : half * 512 + 512], in_=pp[:]
                            )
                        tok0 = q0 + st * 128
                        # output rides the GpSimd (SWDGE) queue, away from the
                        # x-tile prefetches on the sync ring
                        nc.gpsimd.dma_start(out[tok0 : tok0 + 128, :], osb[:])

                    return run

                for st in range(TOK_TILE // 128):
                    filler.append(unit(st))

            def load_x_tile(b, tt4):
                t0 = b * N + tt4 * TOK_TILE
                xts = []
                for half in range(2):
                    xt = xin.tile([128, KO // 2, TOK_TILE], F16, tag="xt")
                    nc.sync.dma_start(
                        xt[:], xT3[:, half * 4 : half * 4 + 4, t0 : t0 + TOK_TILE]
                    )
                    xts.append(xt)
                return xts

            def push_qkv(b, tt4, xts):
                """qkv projection + v transposition units for one 512-token
                tile. q, k, v are computed sequentially so only one PSUM
                accumulator (one avps buf) is live at a time."""
                t0 = b * N + tt4 * TOK_TILE

                def xchunk(ko):
                    return xts[ko // 4][:, ko % 4, :]

                state = {}

                def mm_unit(blk, ko_pair, tag):
                    def run():
                        if ko_pair == 0:
                            state[blk] = avps.tile([128, TOK_TILE], F32, tag=tag, name="qkvps")
                        ps = state[blk]
                        for ko in (2 * ko_pair, 2 * ko_pair + 1):
                            nc.tensor.matmul(
                                ps[:],
                                lhsT=wT_sb[:, ko, blk * 128 : blk * 128 + 128],
                                rhs=xchunk(ko),
                                start=(ko == 0),
                                stop=(ko == KO - 1),
                            )

                    return run

                def add_unit(blk):
                    def run():
                        ps = state.pop(blk)
                        if blk == 0:
                            nc.vector.tensor_scalar_add(
                                qT[:, t0 : t0 + TOK_TILE], ps[:], bias_sb[:, 0:1]
                            )
                        elif blk == 1:
                            nc.vector.tensor_scalar_add(
                                kT[:, t0 : t0 + TOK_TILE], ps[:], bias_sb[:, 1:2]
                            )
                        else:
                            vst = vstage_pool.tile([128, TOK_TILE], F32, tag="vst")
                            nc.vector.tensor_scalar_add(
                                vst[:], ps[:], bias_sb[:, 2:3]
                            )
                            state["vst"] = vst

                    return run

                def tr_unit(st):
                    def run():
                        vst = state["vst"]
                        mc_idx = tt4 * (TOK_TILE // 128) + st
                        pst = avps.tile(
                            [128, 512], F32, tag="avB" if st % 2 == 0 else "avA"
                        )
                        nc.tensor.transpose(
                            pst[:, 0:128], vst[:, st * 128 : st * 128 + 128], ident[:]
                        )
                        nc.vector.tensor_copy(
                            out=v_aug[:, b, mc_idx, 0:64], in_=pst[:, 0:64]
                        )
                        nc.vector.tensor_copy(
                            out=v_aug[:, b, mc_idx, 66:130], in_=pst[:, 64:128]
                        )

                    return run

                for blk, tag in ((0, "avA"), (1, "avB"), (2, "avA")):
                    for kp in range(KO // 2):
                        filler.append(mm_unit(blk, kp, tag))
                    filler.append(add_unit(blk))
                for st in range(TOK_TILE // 128):
                    filler.append(tr_unit(st))

            pending_proj = []

            # --- batch-0 qkv prologue (standalone, sps banks) ---
            for tt4 in range(NT):
                xts = load_x_tile(0, tt4)

                def xchunk(ko, xts=xts):
                    return xts[ko // 4][:, ko % 4, :]

                ps_qk = sps.tile([128, 2 * TOK_TILE], F32, tag="sp")
                t0 = tt4 * TOK_TILE
                for ko in range(KO):
                    for blk in range(2):
                        nc.tensor.matmul(
                            ps_qk[:, blk * TOK_TILE : (blk + 1) * TOK_TILE],
                            lhsT=wT_sb[:, ko, blk * 128 : blk * 128 + 128],
                            rhs=xchunk(ko),
                            start=(ko == 0),
                            stop=(ko == KO - 1),
                        )
                nc.vector.tensor_scalar_add(
                    qT[:, t0 : t0 + TOK_TILE], ps_qk[:, 0:TOK_TILE], bias_sb[:, 0:1]
                )
                nc.vector.tensor_scalar_add(
                    kT[:, t0 : t0 + TOK_TILE],
                    ps_qk[:, TOK_TILE : 2 * TOK_TILE],
                    bias_sb[:, 1:2],
                )
                ps_v = sps.tile([128, 2 * TOK_TILE], F32, tag="sp")
                for ko in range(KO):
                    nc.tensor.matmul(
                        ps_v[:, 0:TOK_TILE],
                        lhsT=wT_sb[:, ko, 256 : 256 + 128],
                        rhs=xchunk(ko),
                        start=(ko == 0),
                        stop=(ko == KO - 1),
                    )
                vst = vstage_pool.tile([128, TOK_TILE], F32, tag="vst")
                nc.vector.tensor_scalar_add(vst[:], ps_v[:, 0:TOK_TILE], bias_sb[:, 2:3])
                for st in range(TOK_TILE // 128):
                    mc_idx = tt4 * (TOK_TILE // 128) + st
                    pst = avps.tile(
                        [128, 512], F32, tag="avB" if st % 2 == 0 else "avA"
                    )
                    nc.tensor.transpose(
                        pst[:, 0:128], vst[:, st * 128 : st * 128 + 128], ident[:]
                    )
                    nc.vector.tensor_copy(out=v_aug[:, 0, mc_idx, 0:64], in_=pst[:, 0:64])
                    nc.vector.tensor_copy(
                        out=v_aug[:, 0, mc_idx, 66:130], in_=pst[:, 64:128]
                    )

            # --- attention blocks ---
            for b in range(B):
                for nt in range(NT):
                    # queue this block's filler: previous block's projection,
                    # then the next batch's qkv tile (x loads issued now so the
                    # DMA lands during the mc loop)
                    while pending_proj:
                        push_proj(*pending_proj.pop(0))
                    if b + 1 < B:
                        push_qkv(b + 1, nt, load_x_tile(b + 1, nt))

                    q0 = b * N + nt * TOK_TILE  # global query offset
                    avA = avps.tile([65, TOK_TILE], F32, tag="avA")
                    avB = avps.tile([65, TOK_TILE], F32, tag="avB")
                    for mc in range(MC):
                        m0 = b * N + mc * 128  # global key offset
                        sp = sps.tile([128, 2 * TOK_TILE], F32, tag="sp")
                        nc.tensor.matmul(
                            sp[:, 0:TOK_TILE],
                            lhsT=kT[0:64, m0 : m0 + 128],
                            rhs=qT[0:64, q0 : q0 + TOK_TILE],
                            start=True,
                            stop=True,
                        )
                        nc.tensor.matmul(
                            sp[:, TOK_TILE : 2 * TOK_TILE],
                            lhsT=kT[64:128, m0 : m0 + 128],
                            rhs=qT[64:128, q0 : q0 + TOK_TILE],
                            start=True,
                            stop=True,
                        )
                        pt = ptpool.tile([128, 2 * TOK_TILE], F16, tag="pt")
                        nc.scalar.activation(
                            pt[:], sp[:], mybir.ActivationFunctionType.Exp
                        )
                        nc.tensor.matmul(
                            avA[:],
                            lhsT=v_aug[:, b, mc, 0:65],
                            rhs=pt[:, 0:TOK_TILE],
                            start=(mc == 0),
                            stop=(mc == MC - 1),
                        )
                        nc.tensor.matmul(
                            avB[:],
                            lhsT=v_aug[:, b, mc, 66:131],
                            rhs=pt[:, TOK_TILE : 2 * TOK_TILE],
                            start=(mc == 0),
                            stop=(mc == MC - 1),
                        )
                        drain(2)
                    drain_all()

                    # stage AV out of PSUM (frees the banks for the next
                    # block), then normalize: reciprocal of the denominator
                    # rows, gpsimd broadcast to 64 partitions, multiply.
                    avstA = small.tile([65, TOK_TILE], F32, tag="avstA")
                    avstB = small.tile([65, TOK_TILE], F32, tag="avstB")
                    nc.vector.tensor_copy(out=avstA[:], in_=avA[:])
                    nc.vector.tensor_copy(out=avstB[:], in_=avB[:])
                    rA0 = small.tile([1, TOK_TILE], F32, tag="rA0")
                    rB0 = small.tile([1, TOK_TILE], F32, tag="rB0")
                    nc.vector.reciprocal_approx_fast(rA0[:], avstA[64:65, :])
                    nc.vector.reciprocal_approx_fast(rB0[:], avstB[64:65, :])
                    rbA = small.tile([64, TOK_TILE], F32, tag="rbA")
                    rbB = small.tile([64, TOK_TILE], F32, tag="rbB")
                    nc.gpsimd.partition_broadcast(rbA[:], rA0[:])
                    nc.gpsimd.partition_broadcast(rbB[:], rB0[:])
                    attn_w = attnpool.tile([128, TOK_TILE], F16, tag="attnw")
                    nc.vector.tensor_tensor(
                        attn_w[0:64, :], avstA[0:64, :], rbA[:], mybir.AluOpType.mult
                    )
                    nc.vector.tensor_tensor(
                        attn_w[64:128, :], avstB[0:64, :], rbB[:], mybir.AluOpType.mult
                    )
                    pending_proj.append((attn_w, q0))

            while pending_proj:
                push_proj(*pending_proj.pop(0))
            drain_all()

    nc.compile()
    return nc


_NC = None


def _get_nc():
    global _NC
    if _NC is None:
        _NC = build_program()
    return _NC


def make_in_maps(x, w_qkv, b_qkv, w_proj):
    x = np.asarray(x, dtype=np.float32)
    w_qkv = np.asarray(w_qkv, dtype=np.float32)
    b_qkv = np.asarray(b_qkv, dtype=np.float32)
    w_proj = np.asarray(w_proj, dtype=np.float32)

    xT = np.ascontiguousarray(x.reshape(BN, C).T).astype(np.float16)  # [C, BN]
    in_maps = []
    for c in range(NCORES):
        r0 = c * CHPC
        wq = w_qkv[r0 : r0 + CHPC] * SCALE
        wk = w_qkv[C + r0 : C + r0 + CHPC]
        wv = w_qkv[2 * C + r0 : 2 * C + r0 + CHPC]
        wT_c = np.ascontiguousarray(
            np.concatenate([wq, wk, wv], axis=0).T
        ).astype(np.float16)
        b_c = np.concatenate(
            [
                b_qkv[r0 : r0 + CHPC] * SCALE,
                b_qkv[C + r0 : C + r0 + CHPC],
                b_qkv[2 * C + r0 : 2 * C + r0 + CHPC],
            ]
        ).astype(np.float32)
        wpT_c = np.ascontiguousarray(w_proj[:, r0 : r0 + CHPC].T).astype(
            np.float16
        )  # [CHPC, C]
        in_maps.append({"xT": xT, "wT": wT_c, "bqkv": b_c, "wpT": wpT_c})
    return in_maps


def kernel(x, w_qkv, b_qkv, w_proj, b_proj, _trace=False, _trace_kwargs=None):
    nc = _get_nc()
    in_maps = make_in_maps(x, w_qkv, b_qkv, w_proj)
    kwargs = {}
    if _trace:
        kwargs.update(trace=True, **(_trace_kwargs or {}))
    res = run_bass_kernel_spmd(nc, in_maps, core_ids=list(range(NCORES)), **kwargs)
    acc = res.results[0]["out"].astype(np.float32)
    for c in range(1, NCORES):
        acc = acc + res.results[c]["out"]
    acc = acc + np.asarray(b_proj, dtype=np.float32)[None, :]
    out = acc.reshape(B, N, C)
    kernel.last_results = res
    return out
